# revision 11
# baseline (speedup 1.0000x reference)
"""Trainium2 Bass kernel for the HMN ragged-sequence model.

Math notes (why this is so much smaller than the reference graph):
  alpha = softmax(M, axis=2) and avg_alpha = alpha.mean(axis=2) -- the mean of
  a softmax over its own normalization axis is exactly 1/T = 1/128 for every
  (b, s).  Likewise avg_beta = 1/S.  Therefore
    avg_alpha == full((B, S, 1), 1/128)                  (input-independent)
    fact[b]   == (129/128) * mean_s(text_emb[b])         (masked segments only)
    label     == (129/128) * mean_t(law_emb)             (same for every b)
  and the whole M/softmax block drops out.  What remains is the memory-bound
  embedding gather plus a tiny FC head.

Device strategy (per core, 8 of them, data-parallel over batch):
  - Host assigns 8 samples/core (LPT on text_lens) and builds a flat stream of
    embedding-row fetches: every valid-segment word id (slot = local sample
    index) plus the 4096 law word ids (slot 8).  Rows are bucketed by
    id >> 15 so each bucket's local index fits dma_gather's int16 contract;
    the row order is irrelevant because each row carries its own weight.
  - One dma_gather per bucket pulls bf16 rows (padded to 512 B on host) into
    SBUF tiles [128, T_k, 256]; row j of a bucket lands at [j%128, j//128].
  - Per 128-row tile, one bf16 matmul  wsel[128,9]^T @ rows[128,200]
    accumulates (scaled, masked) sample sums + the label sum in one PSUM bank.
  - FC head: transpose slots -> feat^T [400, 8], two small matmul stacks with
    W1/W2 (+bias via a 1-row ones matmul), relu, softmax -> predict [8, 6].
"""

import os

import numpy as np
import ml_dtypes

VOCAB = 339503
D = 200
EPAD = 256  # bf16 elements per padded table row (512 B, dma_gather needs %256B)
B, S, W = 64, 128, 32
T_LAW, WL = 128, 32
N_CORES = 8
SLOTS = 9  # 8 sample slots + 1 label slot
C_SCALE = 129.0 / (128.0 * 128.0 * 32.0)
P = 128
BUCKET = 1 << 15  # dma_gather indices are int16
N_BUCKETS = -(-VOCAB // BUCKET)

_prog_cache: dict[tuple, object] = {}
_last_results = None  # test.py introspects this for profiling info


def _build_program(bucket_tiles: tuple[int, ...]):
    """Build + compile the SPMD Bass program.

    bucket_tiles[k] = number of 128-row gather tiles for vocab bucket k
    (identical across cores; cores pad with zero-weight rows).
    """
    import concourse.bacc as bacc
    import concourse.tile as tile
    from concourse import mybir
    from concourse.library_config import mlp

    f32 = mybir.dt.float32
    bf16 = mybir.dt.bfloat16
    i16 = mybir.dt.int16

    n_tiles = sum(bucket_tiles)
    debug_outputs = bool(os.environ.get("BASS_DEBUG_OUTPUTS"))
    nc = bacc.Bacc(
        "TRN2", debug=False, target_bir_lowering=False, num_swdge_queues=4
    )

    emb = nc.dram_tensor("emb", (VOCAB, EPAD), bf16, kind="ExternalInput")
    idx = nc.dram_tensor("idx", (P, 8 * n_tiles), i16, kind="ExternalInput")
    wsel = nc.dram_tensor("wsel", (P, n_tiles, SLOTS), bf16, kind="ExternalInput")
    w1 = nc.dram_tensor("w1", (2 * D, 2 * D), f32, kind="ExternalInput")
    b1 = nc.dram_tensor("b1", (2 * D, 1), f32, kind="ExternalInput")
    w2 = nc.dram_tensor("w2", (2 * D, 6), f32, kind="ExternalInput")
    b2 = nc.dram_tensor("b2", (1, 6), f32, kind="ExternalInput")
    ident = nc.dram_tensor("ident", (SLOTS, SLOTS), f32, kind="ExternalInput")
    predict = nc.dram_tensor("predict", (8, 6), f32, kind="ExternalOutput")
    if debug_outputs:
        dbg_slots = nc.dram_tensor("dbg_slots", (SLOTS, D), f32, kind="ExternalOutput")

    # feature-dim chunking shared by feat^T pieces and W1/W2 row loads
    fchunks = [(0, 128), (128, 200), (200, 328), (328, 400)]  # feat dim (p)
    hchunks = [(0, 128), (128, 256), (256, 384), (384, 400)]  # hidden dim (i)

    with tile.TileContext(nc) as tc:
        with (
            tc.tile_pool(name="sb", bufs=1) as sb,
            tc.tile_pool(name="ps", bufs=1, space="PSUM") as ps,
        ):
            nc.gpsimd.load_library(mlp)
            wsel_sb = sb.tile([P, n_tiles, SLOTS], bf16, tag="wsel")
            nc.sync.dma_start(wsel_sb[:], wsel[:])

            # ---- bucketed gathers + fused reduction matmuls ----
            # Per-bucket idx tiles (aligned bases); sub-gathers of <=GT tiles
            # (512 idxs, the HW-proven size); queue round-robin across the 4
            # SWDGE queues so 4 Q7 pairs generate descriptors in parallel.
            GT = 4
            slots_ps = ps.tile([SLOTS, D], f32, tag="slots")
            t0 = 0
            mm_idx = 0
            qn = 0
            for k, tk in enumerate(bucket_tiles):
                if tk == 0:
                    continue
                b0 = k * BUCKET
                b1_ = min(b0 + BUCKET, VOCAB)
                idx_k = sb.tile([P, 8 * tk], i16, tag=f"idx{k}")
                nc.sync.dma_start(idx_k[:], idx[:, 8 * t0 : 8 * (t0 + tk)])
                g = sb.tile([P, tk, EPAD], bf16, tag=f"g{k}")
                for c0 in range(0, tk, GT):
                    c1 = min(c0 + GT, tk)
                    nck = (c1 - c0) * P
                    nc.gpsimd.dma_gather(
                        g[:, c0:c1, :],
                        emb[b0:b1_, :],
                        idx_k[:, 8 * c0 : 8 * c1],
                        nck,
                        nck,
                        EPAD,
                        queue_num=qn,
                    )
                    qn = (qn + 1) % 4
                for tt in range(tk):
                    nc.tensor.matmul(
                        out=slots_ps[:],
                        lhsT=wsel_sb[:, t0 + tt, :],
                        rhs=g[:, tt, 0:D],
                        start=(mm_idx == 0),
                        stop=(mm_idx == n_tiles - 1),
                    )
                    mm_idx += 1
                t0 += tk

            slots_sb = sb.tile([SLOTS, D], f32, tag="slotsb")
            nc.vector.tensor_copy(out=slots_sb[:], in_=slots_ps[:])
            if debug_outputs:
                nc.sync.dma_start(out=dbg_slots[:], in_=slots_sb[:])

            # ---- feat^T = [fact | label-broadcast] as 4 partition chunks ----
            ident_sb = sb.tile([SLOTS, SLOTS], f32, tag="ident")
            nc.sync.dma_start(ident_sb[:], ident[:])
            tp1 = ps.tile([128, SLOTS], f32, tag="tp1")
            nc.tensor.transpose(
                out=tp1[:], in_=slots_sb[:, 0:128], identity=ident_sb[:]
            )
            tp2 = ps.tile([72, SLOTS], f32, tag="tp2")
            nc.tensor.transpose(
                out=tp2[:], in_=slots_sb[:, 128:200], identity=ident_sb[:]
            )
            featT = []
            for name, src, rows in (
                ("fA", tp1[:, 0:8], 128),
                ("fB", tp2[:, 0:8], 72),
            ):
                f = sb.tile([rows, 8], f32, tag=name)
                nc.vector.tensor_copy(out=f[:], in_=src)
                featT.append(f)
            for name, tp, rows in (("fC", tp1, 128), ("fD", tp2, 72)):
                f = sb.tile([rows, 8], f32, tag=name)
                nc.vector.tensor_copy(out=f[:], in_=tp[:, 8:9].to_broadcast([rows, 8]))
                featT.append(f)

            # ---- h^T = relu(W1^T feat^T + b1) ----
            w1_sb = []
            for k, (p0, p1) in enumerate(fchunks):
                t_ = sb.tile([p1 - p0, 2 * D], f32, tag=f"w1{k}")
                nc.sync.dma_start(t_[:], w1[p0:p1, :])
                w1_sb.append(t_)
            b1_sb = []
            for k, (i0, i1) in enumerate(hchunks):
                t_ = sb.tile([i1 - i0, 1], f32, tag=f"b1{k}")
                nc.sync.dma_start(t_[:], b1[i0:i1, :])
                b1_sb.append(t_)

            hT = []
            for k, (i0, i1) in enumerate(hchunks):
                sz = i1 - i0
                h_ps = ps.tile([sz, 8], f32, tag=f"h{k}")
                for pk, (p0, p1) in enumerate(fchunks):
                    nc.tensor.matmul(
                        out=h_ps[:],
                        lhsT=w1_sb[pk][:, i0:i1],
                        rhs=featT[pk][:],
                        start=(pk == 0),
                        stop=(pk == len(fchunks) - 1),
                    )
                h_sb = sb.tile([sz, 8], f32, tag=f"hT{k}")
                nc.scalar.activation(
                    out=h_sb[:],
                    in_=h_ps[:],
                    func=mybir.ActivationFunctionType.Relu,
                    bias=b1_sb[k][:],
                )
                hT.append(h_sb)

            # ---- logits = h @ W2 + b2 (b2 via a 1-row ones matmul) ----
            w2_sb = []
            for k, (i0, i1) in enumerate(hchunks):
                t_ = sb.tile([i1 - i0, 6], f32, tag=f"w2{k}")
                nc.sync.dma_start(t_[:], w2[i0:i1, :])
                w2_sb.append(t_)
            ones_sb = sb.tile([1, 8], f32, tag="ones")
            nc.vector.memset(ones_sb[:], 1.0)
            b2_sb = sb.tile([1, 6], f32, tag="b2sb")
            nc.sync.dma_start(b2_sb[:], b2[:])

            logits_ps = ps.tile([8, 6], f32, tag="logits")
            for k, (h_sb, w2c) in enumerate(zip(hT, w2_sb)):
                nc.tensor.matmul(
                    out=logits_ps[:],
                    lhsT=h_sb[:],
                    rhs=w2c[:],
                    start=(k == 0),
                    stop=False,
                )
            nc.tensor.matmul(
                out=logits_ps[:],
                lhsT=ones_sb[:],
                rhs=b2_sb[:],
                start=False,
                stop=True,
            )

            # ---- softmax over the 6 classes (free dim) ----
            negmax = sb.tile([8, 1], f32, tag="negmax")
            nc.vector.tensor_reduce(
                out=negmax[:],
                in_=logits_ps[:],
                axis=mybir.AxisListType.X,
                op=mybir.AluOpType.max,
                negate=True,
            )
            esum = sb.tile([8, 1], f32, tag="esum")
            e_sb = sb.tile([8, 6], f32, tag="esb")
            nc.scalar.activation(
                out=e_sb[:],
                in_=logits_ps[:],
                func=mybir.ActivationFunctionType.Exp,
                bias=negmax[:],
                accum_out=esum[:],
            )
            rsum = sb.tile([8, 1], f32, tag="rsum")
            nc.vector.reciprocal(out=rsum[:], in_=esum[:])
            pred_sb = sb.tile([8, 6], f32, tag="pred")
            nc.vector.tensor_scalar_mul(pred_sb[:], e_sb[:], rsum[:])
            nc.sync.dma_start(out=predict[:], in_=pred_sb[:])

    nc.compile()
    return nc


def _assign_cores(lens: np.ndarray):
    """LPT: 8 samples per core, balancing total valid segments."""
    order = np.argsort(-lens, kind="stable")
    loads = [0] * N_CORES
    members: list[list[int]] = [[] for _ in range(N_CORES)]
    for b in order:
        c = min(
            (c for c in range(N_CORES) if len(members[c]) < 8),
            key=lambda c: loads[c],
        )
        members[c].append(int(b))
        loads[c] += int(lens[b])
    return members


def _ensure_axon_hooks_module():
    """bass_utils imports antenv.axon_hooks under trace=True; some images
    lack it entirely (ModuleNotFoundError instead of a graceful skip)."""
    import sys
    import types

    try:
        import antenv.axon_hooks  # noqa: F401
    except ImportError:
        mod = types.ModuleType("antenv.axon_hooks")
        mod._hook = None

        def set_axon_ntff_profile_hook(h):
            mod._hook = h

        def get_axon_ntff_profile_hook():
            return mod._hook

        mod.set_axon_ntff_profile_hook = set_axon_ntff_profile_hook
        mod.get_axon_ntff_profile_hook = get_axon_ntff_profile_hook
        sys.modules["antenv.axon_hooks"] = mod


def _bucketize(ids: np.ndarray, wrow: np.ndarray):
    """Sort a row stream (ids int64, wrow [n] slot ids) into vocab buckets.
    Returns per-bucket (local_ids int16, slots int8) lists."""
    bk = ids >> 15
    order = np.argsort(bk, kind="stable")
    out = []
    ids_s, w_s, bk_s = ids[order], wrow[order], bk[order]
    for k in range(N_BUCKETS):
        m = bk_s == k
        out.append(((ids_s[m] & (BUCKET - 1)).astype(np.int16), w_s[m]))
    return out


def kernel(text_ids, text_lens, law_ids, emb_table, W1, b1, W2, b2):
    global _last_results
    _ensure_axon_hooks_module()
    from concourse import bass_utils

    text_ids = np.asarray(text_ids)
    lens = np.asarray(text_lens).astype(np.int64)
    law_ids = np.asarray(law_ids)
    emb_table = np.asarray(emb_table, dtype=np.float32)
    W1 = np.asarray(W1, dtype=np.float32)
    b1 = np.asarray(b1, dtype=np.float32)
    W2 = np.asarray(W2, dtype=np.float32)
    b2 = np.asarray(b2, dtype=np.float32)

    members = _assign_cores(lens)
    law_flat = law_ids.reshape(-1).astype(np.int64)

    # per-core bucketed row streams
    per_core = []
    for c in range(N_CORES):
        parts_ids = []
        parts_slot = []
        for slot, b in enumerate(members[c]):
            nv = int(lens[b]) * W
            parts_ids.append(text_ids[b, : int(lens[b]), :].reshape(-1))
            parts_slot.append(np.full(nv, slot, np.int8))
        parts_ids.append(law_flat)
        parts_slot.append(np.full(law_flat.size, 8, np.int8))
        ids_c = np.concatenate(parts_ids).astype(np.int64)
        slot_c = np.concatenate(parts_slot)
        per_core.append(_bucketize(ids_c, slot_c))

    bucket_tiles = tuple(
        max(-(-per_core[c][k][0].size // P) for c in range(N_CORES))
        for k in range(N_BUCKETS)
    )
    n_tiles = sum(bucket_tiles)

    emb_pad = np.zeros((VOCAB, EPAD), dtype=ml_dtypes.bfloat16)
    emb_pad[:, :D] = emb_table.astype(ml_dtypes.bfloat16)
    w1_in = np.ascontiguousarray(W1)
    b1_in = np.ascontiguousarray(b1.reshape(2 * D, 1))
    w2_in = np.ascontiguousarray(W2)
    b2_in = np.ascontiguousarray(b2.reshape(1, 6))

    in_maps = []
    for c in range(N_CORES):
        idx16 = np.zeros((P, 8 * n_tiles), np.int16)
        wsel_flat = np.zeros((n_tiles * P, SLOTS), np.float32)
        t0 = 0
        for k, tk in enumerate(bucket_tiles):
            loc, slot = per_core[c][k]
            nk = tk * P
            li = np.zeros(nk, np.int16)
            li[: loc.size] = loc
            blk = li.reshape(-1, 16).T  # [16, 8*tk]; row j -> [j%16, j//16]
            idx16[:, 8 * t0 : 8 * (t0 + tk)] = np.tile(blk, (8, 1))
            rows = t0 * P + np.arange(loc.size)
            wsel_flat[rows, slot.astype(np.int64)] = C_SCALE
            t0 += tk
        wsel3d = np.ascontiguousarray(
            wsel_flat.reshape(n_tiles, P, SLOTS).transpose(1, 0, 2)
        ).astype(ml_dtypes.bfloat16)
        in_maps.append(
            dict(
                emb=emb_pad,
                idx=idx16,
                wsel=wsel3d,
                w1=w1_in,
                b1=b1_in,
                w2=w2_in,
                b2=b2_in,
                ident=np.eye(SLOTS, dtype=np.float32),
            )
        )

    nc = _prog_cache.get(bucket_tiles)
    if nc is None:
        nc = _build_program(bucket_tiles)
        _prog_cache[bucket_tiles] = nc

    res = bass_utils.run_bass_kernel_spmd(
        nc,
        in_maps,
        core_ids=list(range(N_CORES)),
        trace=bool(os.environ.get("BASS_TRACE")),
        tmpdir=os.environ.get("BASS_TMPDIR"),
    )
    _last_results = res

    predict = np.zeros((B, 6), dtype=np.float32)
    for c in range(N_CORES):
        out_c = res.results[c]["predict"]
        for slot, b in enumerate(members[c]):
            predict[b] = out_c[slot]

    avg_alpha = np.full((B, S, 1), 1.0 / T_LAW, dtype=np.float32)
    return predict, avg_alpha


# revision 12
# speedup vs baseline: 1.3904x; 1.3904x over previous
"""Trainium2 Bass kernel for the HMN ragged-sequence model.

Math notes (why this is so much smaller than the reference graph):
  alpha = softmax(M, axis=2) and avg_alpha = alpha.mean(axis=2) -- the mean of
  a softmax over its own normalization axis is exactly 1/T = 1/128 for every
  (b, s).  Likewise avg_beta = 1/S.  Therefore
    avg_alpha == full((B, S, 1), 1/128)                  (input-independent)
    fact[b]   == (129/128) * mean_s(text_emb[b])         (masked segments only)
    label     == (129/128) * mean_t(law_emb)             (same for every b)
  and the whole M/softmax block drops out.  What remains is the memory-bound
  embedding gather plus a tiny FC head.

Device strategy (per core, 8 of them, data-parallel over batch):
  - Host assigns 8 samples/core (LPT on text_lens) and builds a flat stream of
    embedding-row fetches: every valid-segment word id (slot = local sample
    index) plus the 4096 law word ids (slot 8).  Rows are bucketed by
    id >> 15 so each bucket's local index fits dma_gather's int16 contract;
    the row order is irrelevant because each row carries its own weight.
  - One dma_gather per bucket pulls bf16 rows (padded to 512 B on host) into
    SBUF tiles [128, T_k, 256]; row j of a bucket lands at [j%128, j//128].
  - Per 128-row tile, one bf16 matmul  wsel[128,9]^T @ rows[128,200]
    accumulates (scaled, masked) sample sums + the label sum in one PSUM bank.
  - FC head: transpose slots -> feat^T [400, 8], two small matmul stacks with
    W1/W2 (+bias via a 1-row ones matmul), relu, softmax -> predict [8, 6].
"""

import os

import numpy as np
import ml_dtypes

VOCAB = 339503
D = 200
EPAD = 256  # bf16 elements per padded table row (512 B, dma_gather needs %256B)
B, S, W = 64, 128, 32
T_LAW, WL = 128, 32
N_CORES = 8
SLOTS = 9  # 8 sample slots + 1 label slot
C_SCALE = 129.0 / (128.0 * 128.0 * 32.0)
P = 128
BUCKET = 1 << 15  # dma_gather indices are int16
N_BUCKETS = -(-VOCAB // BUCKET)

_prog_cache: dict[tuple, object] = {}
_last_results = None  # test.py introspects this for profiling info


def _build_program(bucket_tiles: tuple[int, ...]):
    """Build + compile the SPMD Bass program.

    bucket_tiles[k] = number of 128-row gather tiles for vocab bucket k
    (identical across cores; cores pad with zero-weight rows).
    """
    import concourse.bacc as bacc
    import concourse.tile as tile
    from concourse import mybir
    from concourse.library_config import mlp

    f32 = mybir.dt.float32
    bf16 = mybir.dt.bfloat16
    i16 = mybir.dt.int16

    n_tiles = sum(bucket_tiles)
    debug_outputs = bool(os.environ.get("BASS_DEBUG_OUTPUTS"))
    nc = bacc.Bacc(
        "TRN2", debug=False, target_bir_lowering=False, num_swdge_queues=4
    )

    emb = nc.dram_tensor("emb", (VOCAB, EPAD), bf16, kind="ExternalInput")
    idx = nc.dram_tensor("idx", (P, 8 * n_tiles), i16, kind="ExternalInput")
    wsel = nc.dram_tensor("wsel", (P, n_tiles, SLOTS), bf16, kind="ExternalInput")
    w1 = nc.dram_tensor("w1", (2 * D, 2 * D), f32, kind="ExternalInput")
    b1 = nc.dram_tensor("b1", (2 * D, 1), f32, kind="ExternalInput")
    w2 = nc.dram_tensor("w2", (2 * D, 6), f32, kind="ExternalInput")
    b2 = nc.dram_tensor("b2", (1, 6), f32, kind="ExternalInput")
    ident = nc.dram_tensor("ident", (SLOTS, SLOTS), f32, kind="ExternalInput")
    predict = nc.dram_tensor("predict", (8, 6), f32, kind="ExternalOutput")
    if debug_outputs:
        dbg_slots = nc.dram_tensor("dbg_slots", (SLOTS, D), f32, kind="ExternalOutput")

    # feature-dim chunking shared by feat^T pieces and W1/W2 row loads
    fchunks = [(0, 128), (128, 200), (200, 328), (328, 400)]  # feat dim (p)
    hchunks = [(0, 128), (128, 256), (256, 384), (384, 400)]  # hidden dim (i)

    with tile.TileContext(nc) as tc:
        with (
            tc.tile_pool(name="sb", bufs=1) as sb,
            tc.tile_pool(name="ps", bufs=1, space="PSUM") as ps,
        ):
            nc.gpsimd.load_library(mlp)
            wsel_sb = sb.tile([P, n_tiles, SLOTS], bf16, tag="wsel")
            nc.sync.dma_start(wsel_sb[:], wsel[:])

            # ---- bucketed gathers + fused reduction matmuls ----
            # Per-bucket idx tiles (aligned bases); sub-gathers of <=GT tiles
            # (512 idxs, the HW-proven size); queue round-robin across the 4
            # SWDGE queues so 4 Q7 pairs generate descriptors in parallel.
            GT = 8
            slots_ps = ps.tile([SLOTS, D], f32, tag="slots")
            t0 = 0
            mm_idx = 0
            qn = 0
            for k, tk in enumerate(bucket_tiles):
                if tk == 0:
                    continue
                b0 = k * BUCKET
                b1_ = min(b0 + BUCKET, VOCAB)
                idx_k = sb.tile([P, 8 * tk], i16, tag=f"idx{k}")
                nc.sync.dma_start(idx_k[:], idx[:, 8 * t0 : 8 * (t0 + tk)])
                g = sb.tile([P, tk, EPAD], bf16, tag=f"g{k}")
                for c0 in range(0, tk, GT):
                    c1 = min(c0 + GT, tk)
                    nck = (c1 - c0) * P
                    nc.gpsimd.dma_gather(
                        g[:, c0:c1, :],
                        emb[b0:b1_, :],
                        idx_k[:, 8 * c0 : 8 * c1],
                        nck,
                        nck,
                        EPAD,
                        queue_num=qn,
                    )
                    qn = (qn + 1) % 4
                for tt in range(tk):
                    nc.tensor.matmul(
                        out=slots_ps[:],
                        lhsT=wsel_sb[:, t0 + tt, :],
                        rhs=g[:, tt, 0:D],
                        start=(mm_idx == 0),
                        stop=(mm_idx == n_tiles - 1),
                    )
                    mm_idx += 1
                t0 += tk

            slots_sb = sb.tile([SLOTS, D], f32, tag="slotsb")
            nc.vector.tensor_copy(out=slots_sb[:], in_=slots_ps[:])
            if debug_outputs:
                nc.sync.dma_start(out=dbg_slots[:], in_=slots_sb[:])

            # ---- feat^T = [fact | label-broadcast] as 4 partition chunks ----
            ident_sb = sb.tile([SLOTS, SLOTS], f32, tag="ident")
            nc.sync.dma_start(ident_sb[:], ident[:])
            tp1 = ps.tile([128, SLOTS], f32, tag="tp1")
            nc.tensor.transpose(
                out=tp1[:], in_=slots_sb[:, 0:128], identity=ident_sb[:]
            )
            tp2 = ps.tile([72, SLOTS], f32, tag="tp2")
            nc.tensor.transpose(
                out=tp2[:], in_=slots_sb[:, 128:200], identity=ident_sb[:]
            )
            featT = []
            for name, src, rows in (
                ("fA", tp1[:, 0:8], 128),
                ("fB", tp2[:, 0:8], 72),
            ):
                f = sb.tile([rows, 8], f32, tag=name)
                nc.vector.tensor_copy(out=f[:], in_=src)
                featT.append(f)
            for name, tp, rows in (("fC", tp1, 128), ("fD", tp2, 72)):
                f = sb.tile([rows, 8], f32, tag=name)
                nc.vector.tensor_copy(out=f[:], in_=tp[:, 8:9].to_broadcast([rows, 8]))
                featT.append(f)

            # ---- h^T = relu(W1^T feat^T + b1) ----
            w1_sb = []
            for k, (p0, p1) in enumerate(fchunks):
                t_ = sb.tile([p1 - p0, 2 * D], f32, tag=f"w1{k}")
                nc.sync.dma_start(t_[:], w1[p0:p1, :])
                w1_sb.append(t_)
            b1_sb = []
            for k, (i0, i1) in enumerate(hchunks):
                t_ = sb.tile([i1 - i0, 1], f32, tag=f"b1{k}")
                nc.sync.dma_start(t_[:], b1[i0:i1, :])
                b1_sb.append(t_)

            hT = []
            for k, (i0, i1) in enumerate(hchunks):
                sz = i1 - i0
                h_ps = ps.tile([sz, 8], f32, tag=f"h{k}")
                for pk, (p0, p1) in enumerate(fchunks):
                    nc.tensor.matmul(
                        out=h_ps[:],
                        lhsT=w1_sb[pk][:, i0:i1],
                        rhs=featT[pk][:],
                        start=(pk == 0),
                        stop=(pk == len(fchunks) - 1),
                    )
                h_sb = sb.tile([sz, 8], f32, tag=f"hT{k}")
                nc.scalar.activation(
                    out=h_sb[:],
                    in_=h_ps[:],
                    func=mybir.ActivationFunctionType.Relu,
                    bias=b1_sb[k][:],
                )
                hT.append(h_sb)

            # ---- logits = h @ W2 + b2 (b2 via a 1-row ones matmul) ----
            w2_sb = []
            for k, (i0, i1) in enumerate(hchunks):
                t_ = sb.tile([i1 - i0, 6], f32, tag=f"w2{k}")
                nc.sync.dma_start(t_[:], w2[i0:i1, :])
                w2_sb.append(t_)
            ones_sb = sb.tile([1, 8], f32, tag="ones")
            nc.vector.memset(ones_sb[:], 1.0)
            b2_sb = sb.tile([1, 6], f32, tag="b2sb")
            nc.sync.dma_start(b2_sb[:], b2[:])

            logits_ps = ps.tile([8, 6], f32, tag="logits")
            for k, (h_sb, w2c) in enumerate(zip(hT, w2_sb)):
                nc.tensor.matmul(
                    out=logits_ps[:],
                    lhsT=h_sb[:],
                    rhs=w2c[:],
                    start=(k == 0),
                    stop=False,
                )
            nc.tensor.matmul(
                out=logits_ps[:],
                lhsT=ones_sb[:],
                rhs=b2_sb[:],
                start=False,
                stop=True,
            )

            # ---- softmax over the 6 classes (free dim) ----
            negmax = sb.tile([8, 1], f32, tag="negmax")
            nc.vector.tensor_reduce(
                out=negmax[:],
                in_=logits_ps[:],
                axis=mybir.AxisListType.X,
                op=mybir.AluOpType.max,
                negate=True,
            )
            esum = sb.tile([8, 1], f32, tag="esum")
            e_sb = sb.tile([8, 6], f32, tag="esb")
            nc.scalar.activation(
                out=e_sb[:],
                in_=logits_ps[:],
                func=mybir.ActivationFunctionType.Exp,
                bias=negmax[:],
                accum_out=esum[:],
            )
            rsum = sb.tile([8, 1], f32, tag="rsum")
            nc.vector.reciprocal(out=rsum[:], in_=esum[:])
            pred_sb = sb.tile([8, 6], f32, tag="pred")
            nc.vector.tensor_scalar_mul(pred_sb[:], e_sb[:], rsum[:])
            nc.sync.dma_start(out=predict[:], in_=pred_sb[:])

    nc.compile()
    return nc


def _assign_cores(lens: np.ndarray):
    """LPT: 8 samples per core, balancing total valid segments."""
    order = np.argsort(-lens, kind="stable")
    loads = [0] * N_CORES
    members: list[list[int]] = [[] for _ in range(N_CORES)]
    for b in order:
        c = min(
            (c for c in range(N_CORES) if len(members[c]) < 8),
            key=lambda c: loads[c],
        )
        members[c].append(int(b))
        loads[c] += int(lens[b])
    return members


def _ensure_axon_hooks_module():
    """bass_utils imports antenv.axon_hooks under trace=True; some images
    lack it entirely (ModuleNotFoundError instead of a graceful skip)."""
    import sys
    import types

    try:
        import antenv.axon_hooks  # noqa: F401
    except ImportError:
        mod = types.ModuleType("antenv.axon_hooks")
        mod._hook = None

        def set_axon_ntff_profile_hook(h):
            mod._hook = h

        def get_axon_ntff_profile_hook():
            return mod._hook

        mod.set_axon_ntff_profile_hook = set_axon_ntff_profile_hook
        mod.get_axon_ntff_profile_hook = get_axon_ntff_profile_hook
        sys.modules["antenv.axon_hooks"] = mod


def _bucketize(ids: np.ndarray, wrow: np.ndarray):
    """Sort a row stream (ids int64, wrow [n] slot ids) into vocab buckets.
    Returns per-bucket (local_ids int16, slots int8) lists."""
    bk = ids >> 15
    order = np.argsort(bk, kind="stable")
    out = []
    ids_s, w_s, bk_s = ids[order], wrow[order], bk[order]
    for k in range(N_BUCKETS):
        m = bk_s == k
        out.append(((ids_s[m] & (BUCKET - 1)).astype(np.int16), w_s[m]))
    return out


def kernel(text_ids, text_lens, law_ids, emb_table, W1, b1, W2, b2):
    global _last_results
    _ensure_axon_hooks_module()
    from concourse import bass_utils

    text_ids = np.asarray(text_ids)
    lens = np.asarray(text_lens).astype(np.int64)
    law_ids = np.asarray(law_ids)
    emb_table = np.asarray(emb_table, dtype=np.float32)
    W1 = np.asarray(W1, dtype=np.float32)
    b1 = np.asarray(b1, dtype=np.float32)
    W2 = np.asarray(W2, dtype=np.float32)
    b2 = np.asarray(b2, dtype=np.float32)

    members = _assign_cores(lens)
    law_flat = law_ids.reshape(-1).astype(np.int64)

    # per-core bucketed row streams
    per_core = []
    for c in range(N_CORES):
        parts_ids = []
        parts_slot = []
        for slot, b in enumerate(members[c]):
            nv = int(lens[b]) * W
            parts_ids.append(text_ids[b, : int(lens[b]), :].reshape(-1))
            parts_slot.append(np.full(nv, slot, np.int8))
        parts_ids.append(law_flat)
        parts_slot.append(np.full(law_flat.size, 8, np.int8))
        ids_c = np.concatenate(parts_ids).astype(np.int64)
        slot_c = np.concatenate(parts_slot)
        per_core.append(_bucketize(ids_c, slot_c))

    bucket_tiles = tuple(
        max(-(-per_core[c][k][0].size // P) for c in range(N_CORES))
        for k in range(N_BUCKETS)
    )
    n_tiles = sum(bucket_tiles)

    emb_pad = np.zeros((VOCAB, EPAD), dtype=ml_dtypes.bfloat16)
    emb_pad[:, :D] = emb_table.astype(ml_dtypes.bfloat16)
    w1_in = np.ascontiguousarray(W1)
    b1_in = np.ascontiguousarray(b1.reshape(2 * D, 1))
    w2_in = np.ascontiguousarray(W2)
    b2_in = np.ascontiguousarray(b2.reshape(1, 6))

    in_maps = []
    for c in range(N_CORES):
        idx16 = np.zeros((P, 8 * n_tiles), np.int16)
        wsel_flat = np.zeros((n_tiles * P, SLOTS), np.float32)
        t0 = 0
        for k, tk in enumerate(bucket_tiles):
            loc, slot = per_core[c][k]
            nk = tk * P
            li = np.zeros(nk, np.int16)
            li[: loc.size] = loc
            blk = li.reshape(-1, 16).T  # [16, 8*tk]; row j -> [j%16, j//16]
            idx16[:, 8 * t0 : 8 * (t0 + tk)] = np.tile(blk, (8, 1))
            rows = t0 * P + np.arange(loc.size)
            wsel_flat[rows, slot.astype(np.int64)] = C_SCALE
            t0 += tk
        wsel3d = np.ascontiguousarray(
            wsel_flat.reshape(n_tiles, P, SLOTS).transpose(1, 0, 2)
        ).astype(ml_dtypes.bfloat16)
        in_maps.append(
            dict(
                emb=emb_pad,
                idx=idx16,
                wsel=wsel3d,
                w1=w1_in,
                b1=b1_in,
                w2=w2_in,
                b2=b2_in,
                ident=np.eye(SLOTS, dtype=np.float32),
            )
        )

    nc = _prog_cache.get(bucket_tiles)
    if nc is None:
        nc = _build_program(bucket_tiles)
        _prog_cache[bucket_tiles] = nc

    res = bass_utils.run_bass_kernel_spmd(
        nc,
        in_maps,
        core_ids=list(range(N_CORES)),
        trace=bool(os.environ.get("BASS_TRACE")),
        tmpdir=os.environ.get("BASS_TMPDIR"),
    )
    _last_results = res

    predict = np.zeros((B, 6), dtype=np.float32)
    for c in range(N_CORES):
        out_c = res.results[c]["predict"]
        for slot, b in enumerate(members[c]):
            predict[b] = out_c[slot]

    avg_alpha = np.full((B, S, 1), 1.0 / T_LAW, dtype=np.float32)
    return predict, avg_alpha
